# revision 5
# baseline (speedup 1.0000x reference)
"""Trainium2 Bass kernel for differentiable rotated-box IoU (DiffIouRotated).

Full inputs: box1, box2 [4, 131072, 5] f32 (x, y, w, h, alpha).
Output: IoU [4, 131072] f32.

Green's-theorem formulation with edge-midpoint parametrization (see
proto_new.py): each of the 8 box edges contributes
(1/4)cross(mid,D)*Δs⁺, where Δs⁺ is the Liang-Barsky-clipped length in
s ∈ [-1,1] against the other box's slab.  Per-edge cross terms reduce to
two cross products plus Δs sums/differences.

Numerics: fp16 everywhere past the trig (DVE 2-byte tensor_tensor runs
the 2x perf mode, tensor_scalar the 4x mode); f32 only for sin/cos and
the base reciprocals 1/cr, 1/sr, 1/w, 1/h (clamped to ±4096 before the
fp16 convert so no inf-inf can arise).  Measured L2 rel err 1.4e-3 on
the reference data (gate 2e-2).

The two clip passes are processed as merged [128 x 8*512] tiles so most
interval-stage instructions cover both passes at once (w=8 ops).

Sharding: data-parallel, 65536 pairs/core as [128 part x 512 free],
field-major SBUF layout.
"""

import os
import sys

import numpy as np

if "/opt/trn_rl_repo" not in sys.path:
    sys.path.insert(0, "/opt/trn_rl_repo")

import concourse.bass as bass
import concourse.bacc as bacc
import concourse.mybir as mybir
from concourse.bass_utils import run_bass_kernel_spmd
from concourse.tile import TileContext

F32 = mybir.dt.float32
F16 = mybir.dt.float16
U16 = mybir.dt.uint16
U32 = mybir.dt.uint32
OP = mybir.AluOpType
AF = mybir.ActivationFunctionType

NCORES = 8
P = 128
S = 65536
F = S // P           # 512
PI = float(np.pi)
RCL = 4096.0         # clamp for 1/sin, 1/cos before fp16 convert

_CACHE = {}
LAST_RESULTS = None


def _build_program():
    nc = bacc.Bacc("TRN2", target_bir_lowering=False, debug=False,
                   num_devices=NCORES)
    _ct = nc.alloc_sbuf_tensor("const-f32-halfpi", [P, 1], F32)
    nc.gpsimd.memset(_ct.ap(), PI / 2)
    nc.const_aps.aps[(F32, PI / 2)] = _ct.ap()
    nc.all_engine_barrier()

    b1 = nc.dram_tensor("b1", [P, 5 * F], F32, kind="ExternalInput")
    b2 = nc.dram_tensor("b2", [P, 5 * F], F32, kind="ExternalInput")
    iou = nc.dram_tensor("iou", [S], F32, kind="ExternalOutput")

    repeat = int(os.environ.get("KREPEAT", "1"))
    with TileContext(nc) as tc:
        with tc.tile_pool(name="pool", bufs=1) as pool:
            if repeat > 1:
                with tc.For_i(0, repeat, 1):
                    _emit(nc, pool, b1, b2, iou)
            else:
                _emit(nc, pool, b1, b2, iou)
    nc.compile()
    return nc


def _ap(t, off, dims):
    return bass.AP(t.tensor, t.offset + off, [t.ap[0]] + dims)


def _emit(nc, pool, b1, b2, iou):
    V, G, A = nc.vector, nc.gpsimd, nc.scalar
    tt = V.tensor_tensor
    ts = V.tensor_scalar

    def T(name, w=1, dt=F16, tag=None):
        return pool.tile([P, w * F], dt, name=name, tag=(tag or name))

    def q(t, r=2):
        return t.rearrange("p (r f) -> p r f", r=r)

    # ---------------- input DMAs ----------------
    TA1, TA2 = T("TA1", 1, F32), T("TA2", 1, F32)
    TX1, TX2 = T("TX1", 4, F32), T("TX2", 4, F32)

    b1v = b1.ap().flatten().rearrange("(p q) -> p q", p=P)
    b2v = b2.ap().flatten().rearrange("(p q) -> p q", p=P)
    nc.sync.dma_start(TA1[:], b1v[:, 4 * F:])
    nc.sync.dma_start(TA2[:], b2v[:, 4 * F:])
    nc.sync.dma_start(TX1[:], b1v[:, :4 * F])
    nc.sync.dma_start(TX2[:], b2v[:, :4 * F])

    # ---------------- trig (Act) ----------------
    DA = T("DA", 1, F32)
    tt(DA[:], TA1[:], TA2[:], OP.subtract)
    ADA = T("ADA", 1, F32)
    A.activation(ADA[:], DA[:], AF.Abs)
    # TRIG4 = [cr | sr | -sr | cr] fp16 ; SC3 = [c2 | s2 | -s2] fp16
    TRIG4 = T("TRIG4", 4)
    cr_both = _ap(TRIG4, 0, [[3 * F, 2], [1, F]])
    A.activation(cr_both, _ap(ADA, 0, [[0, 2], [1, F]]), AF.Sin,
                 bias=PI / 2, scale=-1.0)
    A.activation(TRIG4[:, F:2 * F], DA[:], AF.Sin)
    A.activation(TRIG4[:, 2 * F:3 * F], DA[:], AF.Sin, scale=-1.0)
    SC3 = T("SC3", 3)
    A.activation(SC3[:, :F], TA2[:], AF.Sin, bias=PI / 2, scale=-1.0)
    A.activation(SC3[:, F:2 * F], TA2[:], AF.Sin)
    A.activation(SC3[:, 2 * F:], TA2[:], AF.Sin, scale=-1.0)

    # ---------------- reciprocals (f32 core) ----------------
    TRIGF = T("TRIGF", 2, F32)
    A.activation(TRIGF[:], TRIG4[:, :2 * F], AF.Copy)
    RTRIG = T("RTRIG", 2, F32)
    V.reciprocal_approx_fast(out=RTRIG[:], in_=TRIGF[:])
    # RT4 = [Rcr | Rsr | -Rsr | Rcr] fp16, clamped to +-RCL
    RT4 = T("RT4", 4)
    ts(out=_ap(RT4, 0, [[3 * F, 2], [1, F]]),
       in0=_ap(RTRIG, 0, [[0, 2], [1, F]]), scalar1=RCL, scalar2=-RCL,
       op0=OP.min, op1=OP.max)
    ts(out=RT4[:, F:2 * F], in0=RTRIG[:, F:], scalar1=RCL, scalar2=-RCL,
       op0=OP.min, op1=OP.max)
    ts(out=RT4[:, 2 * F:3 * F], in0=RT4[:, F:2 * F], scalar1=-1.0,
       scalar2=None, op0=OP.mult)
    # RWH16B = [2/w1 | 2/h1 | 2/w2 | 2/h2] (recips of half-extents)
    RWHF = T("RWHF", 4, F32)
    V.reciprocal_approx_fast(out=RWHF[:, :2 * F], in_=TX1[:, 2 * F:])
    V.reciprocal_approx_fast(out=RWHF[:, 2 * F:], in_=TX2[:, 2 * F:])
    RWH16 = T("RWH16", 4)
    ts(out=RWH16[:], in0=RWHF[:], scalar1=2.0, scalar2=None, op0=OP.mult)
    # WH16B = [w1/2 | h1/2 | w2/2 | h2/2] fp16
    WH16 = T("WH16", 4)
    ts(out=WH16[:, :2 * F], in0=TX1[:, 2 * F:], scalar1=0.5, scalar2=None,
       op0=OP.mult)
    ts(out=WH16[:, 2 * F:], in0=TX2[:, 2 * F:], scalar1=0.5, scalar2=None,
       op0=OP.mult)

    # ---------------- transforms (fp16) ----------------
    DXY = T("DXY", 2)
    tt(DXY[:], TX1[:, :2 * F], TX2[:, :2 * F], OP.subtract)  # [dx | dy]
    dxy_sw = _ap(DXY, F, [[-F, 2], [1, F]])                  # [dy | dx]
    # P12 = [c2 dx | s2 dy | c2 dy | -s2 dx]
    P12 = T("P12", 4)
    tt(P12[:, :2 * F], SC3[:, :2 * F], DXY[:], OP.mult)
    tt(q(P12[:, 2 * F:]), _ap(SC3, 0, [[2 * F, 2], [1, F]]), dxy_sw, OP.mult)
    # TC16 = [tx | cx | ty | cy]
    TC16 = T("TC16", 4)
    txty = _ap(TC16, 0, [[2 * F, 2], [1, F]])    # [tx | ty] dest/src view
    tt(txty, _ap(P12, 0, [[2 * F, 2], [1, F]]),
       _ap(P12, F, [[2 * F, 2], [1, F]]), OP.add)
    # P34 = [cr tx | sr ty | cr ty | -sr tx]
    tytx = _ap(TC16, 2 * F, [[-2 * F, 2], [1, F]])  # [ty | tx]
    P34 = T("P34", 4, tag="P12")
    tt(q(P34[:, :2 * F]), q(TRIG4[:, :2 * F]), txty, OP.mult)
    tt(q(P34[:, 2 * F:]), _ap(TRIG4, 0, [[2 * F, 2], [1, F]]), tytx, OP.mult)
    QQ = T("QQ", 2)
    tt(q(QQ[:]), _ap(P34, 0, [[2 * F, 2], [1, F]]),
       _ap(P34, F, [[2 * F, 2], [1, F]]), OP.add)  # [-cx | -cy]
    cxcy = _ap(TC16, F, [[2 * F, 2], [1, F]])    # [cx | cy] dest view
    ts(out=cxcy, in0=q(QQ[:]), scalar1=-1.0, scalar2=None, op0=OP.mult)

    # ---------------- per-pass pairs (merged w=4 ops) ----------------
    # RP = [p1: (rpx pair | rpy pair) | p2: (...)]; rp*-pair = signed 2/D'
    # pass1: rpx = (-2Rh1 Rsr, 2Rw1 Rcr), rpy = (2Rh1 Rcr, 2Rw1 Rsr)
    # pass2: rpx = (2Rh2 Rsr, 2Rw2 Rcr),  rpy = (2Rh2 Rcr, -2Rw2 Rsr)
    RP = T("RP", 8)
    rw1rep = _ap(RWH16, F, [[0, 2], [-F, 2], [1, F]])    # [2/h1 2/w1] x2
    rw2rep = _ap(RWH16, 3 * F, [[0, 2], [-F, 2], [1, F]])
    rt_p1 = _ap(RT4, 2 * F, [[-2 * F, 2], [F, 2], [1, F]])  # [-Rsr Rcr | Rcr Rsr]
    rt_p2 = _ap(RT4, F, [[2 * F, 2], [-F, 2], [1, F]])      # [Rsr Rcr | Rcr -Rsr]
    tt(RP[:, :4 * F].rearrange("p (i j f) -> p i j f", i=2, j=2),
       rw1rep, rt_p1, OP.mult)
    tt(RP[:, 4 * F:].rearrange("p (i j f) -> p i j f", i=2, j=2),
       rw2rep, rt_p2, OP.mult)
    # OFF = [p1: (offx pair | offy pair) | p2: (...)]; half-dir midpoint offsets
    # pass1: offx = (w1/2 cr, -h1/2 sr), offy = (w1/2 sr, h1/2 cr)
    # pass2: offx = (w2/2 cr, h2/2 sr),  offy = (-w2/2 sr, h2/2 cr)
    OFF = T("OFF", 8)
    wh1rep = _ap(WH16, 0, [[0, 2], [F, 2], [1, F]])      # [w1/2 h1/2] x2
    wh2rep = _ap(WH16, 2 * F, [[0, 2], [F, 2], [1, F]])
    tg_p1 = _ap(TRIG4, 0, [[F, 2], [2 * F, 2], [1, F]])  # [cr -sr | sr cr]
    tg_p2 = _ap(TRIG4, 0, [[2 * F, 2], [F, 2], [1, F]])  # [cr sr | -sr cr]
    tt(OFF[:, :4 * F].rearrange("p (i j f) -> p i j f", i=2, j=2),
       wh1rep, tg_p1, OP.mult)
    tt(OFF[:, 4 * F:].rearrange("p (i j f) -> p i j f", i=2, j=2),
       wh2rep, tg_p2, OP.mult)

    # slot layout in [P, 8F] tiles: [p1+(2F) | p2+(2F) | p1-(2F) | p2-(2F)]
    tcx = _ap(TC16, 0, [[F, 2], [0, 2], [1, F]])     # [tx tx cx cx]
    tcy = _ap(TC16, 2 * F, [[F, 2], [0, 2], [1, F]])  # [ty ty cy cy]
    offx4 = _ap(OFF, 0, [[4 * F, 2], [F, 2], [1, F]])      # [offx1 | offx2]
    offy4 = _ap(OFF, 2 * F, [[4 * F, 2], [F, 2], [1, F]])  # [offy1 | offy2]
    UX = T("UX", 8)
    tt(UX[:, :4 * F].rearrange("p (i j f) -> p i j f", i=2, j=2),
       tcx, offx4, OP.add)
    tt(UX[:, 4 * F:].rearrange("p (i j f) -> p i j f", i=2, j=2),
       tcx, offx4, OP.subtract)
    UY = T("UY", 8)
    tt(UY[:, :4 * F].rearrange("p (i j f) -> p i j f", i=2, j=2),
       tcy, offy4, OP.add)
    tt(UY[:, 4 * F:].rearrange("p (i j f) -> p i j f", i=2, j=2),
       tcy, offy4, OP.subtract)
    # mneg = u * r  (r repeats over the +/- halves)
    rpx_rep = _ap(RP, 0, [[0, 2], [4 * F, 2], [1, 2 * F]])
    rpy_rep = _ap(RP, 2 * F, [[0, 2], [4 * F, 2], [1, 2 * F]])
    MX = T("MX", 8)
    tt(MX[:].rearrange("p (r s f) -> p r s f", r=2, s=2),
       UX[:].rearrange("p (r s f) -> p r s f", r=2, s=2), rpx_rep, OP.mult)
    MY = T("MY", 8)
    tt(MY[:].rearrange("p (r s f) -> p r s f", r=2, s=2),
       UY[:].rearrange("p (r s f) -> p r s f", r=2, s=2), rpy_rep, OP.mult)
    # slab half-widths: sigma = W''_other * |r|
    w2rep = _ap(WH16, 2 * F, [[-2 * F, 2], [0, 2], [1, F]])  # [w2/2 w2/2 w1/2 w1/2]
    h2rep = _ap(WH16, 3 * F, [[-2 * F, 2], [0, 2], [1, F]])  # [h2/2 h2/2 h1/2 h1/2]
    rpx_pt = _ap(RP, 0, [[4 * F, 2], [F, 2], [1, F]])
    rpy_pt = _ap(RP, 2 * F, [[4 * F, 2], [F, 2], [1, F]])
    PTX = T("PTX", 4)
    tt(PTX[:].rearrange("p (i j f) -> p i j f", i=2, j=2), w2rep,
       rpx_pt, OP.mult)
    PTY = T("PTY", 4)
    tt(PTY[:].rearrange("p (i j f) -> p i j f", i=2, j=2), h2rep,
       rpy_pt, OP.mult)
    SQX = T("SQX", 4, tag="OFF")
    ts(out=SQX[:].bitcast(U16), in0=PTX[:].bitcast(U16), scalar1=0x7FFF,
       scalar2=None, op0=OP.bitwise_and)
    SQY = T("SQY", 4, tag="RP")
    ts(out=SQY[:].bitcast(U16), in0=PTY[:].bitcast(U16), scalar1=0x7FFF,
       scalar2=None, op0=OP.bitwise_and)
    # combos: ax = sigma - mneg, bx = sigma + mneg
    sqx_rep = _ap(SQX, 0, [[0, 2], [1, 4 * F]])
    sqy_rep = _ap(SQY, 0, [[0, 2], [1, 4 * F]])
    AX = T("AX", 8, tag="UX")
    tt(q(AX[:]), sqx_rep, q(MX[:]), OP.subtract)
    BX = T("BX", 8)
    tt(q(BX[:]), sqx_rep, q(MX[:]), OP.add)
    AY = T("AY", 8, tag="UY")
    tt(q(AY[:]), sqy_rep, q(MY[:]), OP.subtract)
    BY = T("BY", 8, tag="MX")
    tt(q(BY[:]), sqy_rep, q(MY[:]), OP.add)
    NL = T("NL", 8, tag="MY")
    tt(NL[:], AX[:], AY[:], OP.min)
    HI = T("HI", 8, tag="UX")
    tt(HI[:], BX[:], BY[:], OP.min)
    NLC = T("NLC", 8, tag="BX")
    ts(out=NLC[:], in0=NL[:], scalar1=1.0, scalar2=None, op0=OP.min)
    HIC = T("HIC", 8, tag="UY")
    ts(out=HIC[:], in0=HI[:], scalar1=1.0, scalar2=None, op0=OP.min)
    DS = T("DS", 8, tag="MY")
    tt(DS[:], NLC[:], HIC[:], OP.add)
    RDT = T("RDT", 8, tag="UX")
    ts(out=RDT[:], in0=DS[:], scalar1=0.0, scalar2=None, op0=OP.max)

    # ---------------- combine ----------------
    # RDT slots: [w+1 h+1 (2F) | w+2 h+2 | w-1 h-1 | w-2 h-2]
    # d_w(p1) = rdt[w+1] - rdt[w-1]; d_h(p1) = rdt[h-1] - rdt[h+1]
    ddA = _ap(RDT, 0, [[5 * F, 2], [1, F]])      # [w+1 | h-1]
    ddB = _ap(RDT, 4 * F, [[-3 * F, 2], [1, F]])  # [w-1 | h+1]
    DD = T("DD", 2)
    tt(q(DD[:]), ddA, ddB, OP.subtract)
    # S1 = sum of pass1 slots, S2 = sum of pass2 slots
    SSQ = T("SSQ", 4, tag="BX")
    G.tensor_tensor(q(SSQ[:], 2).rearrange("p r (j f) -> p r j f", j=2),
       q(RDT[:, :4 * F], 2).rearrange("p r (j f) -> p r j f", j=2),
       q(RDT[:, 4 * F:], 2).rearrange("p r (j f) -> p r j f", j=2), OP.add)
    ssA = _ap(SSQ, 0, [[2 * F, 2], [1, F]])
    ssB = _ap(SSQ, F, [[2 * F, 2], [1, F]])
    S12 = T("S12", 2)
    G.tensor_tensor(q(S12[:]), ssA, ssB, OP.add)              # [S1 | S2]
    # cross pair: [crh/2 | -crw/2] = [h1/2 * QQ0 | w1/2 * QQ1]
    wh_sw1 = _ap(WH16, F, [[-F, 2], [1, F]])     # [h1/2 | w1/2]
    CRP = T("CRP", 2, tag="P1")
    tt(q(CRP[:]), wh_sw1, q(QQ[:]), OP.mult)
    XX = T("XX", 2, tag="P2")
    tt(XX[:], CRP[:], DD[:], OP.mult)            # [crh/2 d_w | -crw/2 d_h]
    XD = T("XD", 1)
    tt(XD[:], XX[:, :F], XX[:, F:], OP.subtract)  # G1/2
    # WHP = [w1 h1 / 4 | w2 h2 / 4]
    whA = _ap(WH16, 0, [[2 * F, 2], [1, F]])
    whB = _ap(WH16, F, [[2 * F, 2], [1, F]])
    WHP = T("WHP", 2)
    G.tensor_tensor(q(WHP[:]), whA, whB, OP.mult)
    TTm = T("TTm", 2, tag="QQ")
    G.tensor_tensor(TTm[:], WHP[:], S12[:], OP.mult)          # [wh1 S1 / 4 | wh2 S2 / 4]
    HS4 = T("HS4", 1, tag="DD")
    G.tensor_tensor(HS4[:], TTm[:, :F], TTm[:, F:], OP.add)   # (wh1 S1 + wh2 S2)/4
    GH = T("GH", 1, tag="S12")
    tt(GH[:], XD[:], HS4[:], OP.add)             # G/2
    ABF = T("ABF", 1)
    ts(out=ABF[:].bitcast(U16), in0=GH[:].bitcast(U16), scalar1=0x7FFF,
       scalar2=None, op0=OP.bitwise_and)         # |G|/2 = 2*area, fp16
    SW = T("SW", 1, tag="XD")
    G.tensor_tensor(SW[:], WHP[:, :F], WHP[:, F:], OP.add)    # (wh1+wh2)/4
    SW8 = T("SW8", 1, F32)
    A.activation(SW8[:], SW[:], AF.Copy, scale=8.0)
    U4 = T("U4", 1, F32)
    tt(U4[:], SW8[:], ABF[:], OP.subtract)       # 2(a1+a2) - 2*area
    RC = T("RC", 1, F32)
    V.reciprocal_approx_fast(out=RC[:], in_=U4[:])
    IOU = T("IOU", 1, F32)
    tt(IOU[:], ABF[:], RC[:], OP.mult)
    ioudst = bass.AP(iou.ap().tensor, iou.ap().offset, [[F, P], [1, F]])
    nc.sync.dma_start(ioudst, IOU[:])


def _get_program():
    key = ("prog", os.environ.get("KREPEAT", "1"))
    if key not in _CACHE:
        _CACHE[key] = _build_program()
    return _CACHE[key]


def kernel(box1, box2, trace=False):
    global LAST_RESULTS
    b1 = np.asarray(box1, dtype=np.float32)
    b2 = np.asarray(box2, dtype=np.float32)
    B, N, C = b1.shape
    Tn = B * N
    assert Tn == NCORES * S and C == 5, (b1.shape,)
    b1f = np.ascontiguousarray(
        b1.reshape(NCORES, P, F, 5).transpose(0, 1, 3, 2)).reshape(NCORES, P, 5 * F)
    b2f = np.ascontiguousarray(
        b2.reshape(NCORES, P, F, 5).transpose(0, 1, 3, 2)).reshape(NCORES, P, 5 * F)
    in_maps = [{"b1": b1f[i], "b2": b2f[i]} for i in range(NCORES)]
    nc = _get_program()
    res = run_bass_kernel_spmd(nc, in_maps, list(range(NCORES)), trace=trace)
    LAST_RESULTS = res
    out = np.concatenate([res.results[i]["iou"] for i in range(NCORES)])
    return out.reshape(B, N)


if __name__ == "__main__":
    from concourse.bass_interp import CoreSim

    nc = _get_program()
    print("program built; instructions:",
          sum(len(bb.instructions) for bb in nc.main_func.blocks))
    d = np.load(os.path.join(os.path.dirname(os.path.abspath(__file__)),
                             "ref_cache.npz"))
    b1 = d["box1"].reshape(-1, 5)[:S]
    b2 = d["box2"].reshape(-1, 5)[:S]
    exp = d["expected"].reshape(-1)[:S]
    sim = CoreSim(nc)
    sim.tensor("b1")[:] = np.ascontiguousarray(
        b1.reshape(P, F, 5).transpose(0, 2, 1)).reshape(P, 5 * F)
    sim.tensor("b2")[:] = np.ascontiguousarray(
        b2.reshape(P, F, 5).transpose(0, 2, 1)).reshape(P, 5 * F)
    sim.simulate()
    got = np.array(sim.tensor("iou"))
    err = got - exp
    l2 = np.linalg.norm(err) / np.linalg.norm(exp)
    print(f"CoreSim vs reference: L2rel {l2:.3e} maxabs {np.abs(err).max():.3e} "
          f"nan {np.isnan(got).sum()}")


# revision 6
# speedup vs baseline: 1.0878x; 1.0878x over previous
"""Trainium2 Bass kernel for differentiable rotated-box IoU (DiffIouRotated).

Full inputs: box1, box2 [4, 131072, 5] f32 (x, y, w, h, alpha).
Output: IoU [4, 131072] f32.

Green's-theorem formulation with edge-midpoint parametrization (see
proto_new.py): each of the 8 box edges contributes
(1/4)cross(mid,D)*Δs⁺, where Δs⁺ is the Liang-Barsky-clipped length in
s ∈ [-1,1] against the other box's slab.  Per-edge cross terms reduce to
two cross products plus Δs sums/differences.

Numerics: fp16 everywhere past the trig (DVE 2-byte tensor_tensor runs
the 2x perf mode, tensor_scalar the 4x mode); f32 only for sin/cos and
the base reciprocals 1/cr, 1/sr, 1/w, 1/h (clamped to ±4096 before the
fp16 convert so no inf-inf can arise).  Measured L2 rel err 1.4e-3 on
the reference data (gate 2e-2).

The two clip passes are processed as merged [128 x 8*512] tiles so most
interval-stage instructions cover both passes at once (w=8 ops).

Sharding: data-parallel, 65536 pairs/core as [128 part x 512 free],
field-major SBUF layout.
"""

import os
import sys

import numpy as np

if "/opt/trn_rl_repo" not in sys.path:
    sys.path.insert(0, "/opt/trn_rl_repo")

import concourse.bass as bass
import concourse.bacc as bacc
import concourse.mybir as mybir
from concourse.bass_utils import run_bass_kernel_spmd
from concourse.tile import TileContext

F32 = mybir.dt.float32
F16 = mybir.dt.float16
U16 = mybir.dt.uint16
U32 = mybir.dt.uint32
OP = mybir.AluOpType
AF = mybir.ActivationFunctionType

NCORES = 8
P = 128
S = 65536
F = S // P           # 512
PI = float(np.pi)
RCL = 4096.0         # clamp for 1/sin, 1/cos before fp16 convert

_CACHE = {}
LAST_RESULTS = None


def _build_program():
    nc = bacc.Bacc("TRN2", target_bir_lowering=False, debug=False,
                   num_devices=NCORES)
    _ct = nc.alloc_sbuf_tensor("const-f32-halfpi", [P, 1], F32)
    nc.gpsimd.memset(_ct.ap(), PI / 2)
    nc.const_aps.aps[(F32, PI / 2)] = _ct.ap()
    nc.all_engine_barrier()

    b1 = nc.dram_tensor("b1", [P, 5 * F], F32, kind="ExternalInput")
    b2 = nc.dram_tensor("b2", [P, 5 * F], F32, kind="ExternalInput")
    iou = nc.dram_tensor("iou", [S], F32, kind="ExternalOutput")

    repeat = int(os.environ.get("KREPEAT", "1"))
    with TileContext(nc) as tc:
        with tc.tile_pool(name="pool", bufs=1) as pool:
            if repeat > 1:
                with tc.For_i(0, repeat, 1):
                    _emit(nc, pool, b1, b2, iou)
            else:
                _emit(nc, pool, b1, b2, iou)
    nc.compile()
    return nc


def _ap(t, off, dims):
    return bass.AP(t.tensor, t.offset + off, [t.ap[0]] + dims)


def _emit(nc, pool, b1, b2, iou):
    V, G, A = nc.vector, nc.gpsimd, nc.scalar
    tt = V.tensor_tensor
    ts = V.tensor_scalar

    def T(name, w=1, dt=F16, tag=None):
        return pool.tile([P, w * F], dt, name=name, tag=(tag or name))

    def q(t, r=2):
        return t.rearrange("p (r f) -> p r f", r=r)

    # ---------------- input DMAs ----------------
    TA1, TA2 = T("TA1", 1, F32), T("TA2", 1, F32)
    TX1, TX2 = T("TX1", 4, F32), T("TX2", 4, F32)

    b1v = b1.ap().flatten().rearrange("(p q) -> p q", p=P)
    b2v = b2.ap().flatten().rearrange("(p q) -> p q", p=P)
    nc.sync.dma_start(TA1[:], b1v[:, 4 * F:])
    nc.sync.dma_start(TA2[:], b2v[:, 4 * F:])
    nc.sync.dma_start(TX1[:], b1v[:, :4 * F])
    nc.sync.dma_start(TX2[:], b2v[:, :4 * F])

    # ---------------- trig (Act) ----------------
    DA = T("DA", 1, F32)
    tt(DA[:], TA1[:], TA2[:], OP.subtract)
    ADA = T("ADA", 1, F32)
    A.activation(ADA[:], DA[:], AF.Abs)
    # TRIG4 = [cr | sr | -sr | cr] fp16 ; SC3 = [c2 | s2 | -s2] fp16
    TRIG4 = T("TRIG4", 4)
    cr_both = _ap(TRIG4, 0, [[3 * F, 2], [1, F]])
    A.activation(cr_both, _ap(ADA, 0, [[0, 2], [1, F]]), AF.Sin,
                 bias=PI / 2, scale=-1.0)
    A.activation(TRIG4[:, F:2 * F], DA[:], AF.Sin)
    A.activation(TRIG4[:, 2 * F:3 * F], DA[:], AF.Sin, scale=-1.0)
    SC3 = T("SC3", 3)
    A.activation(SC3[:, :F], TA2[:], AF.Sin, bias=PI / 2, scale=-1.0)
    A.activation(SC3[:, F:2 * F], TA2[:], AF.Sin)
    A.activation(SC3[:, 2 * F:], TA2[:], AF.Sin, scale=-1.0)

    # ---------------- reciprocals (f32 core) ----------------
    TRIGF = T("TRIGF", 2, F32)
    A.activation(TRIGF[:], TRIG4[:, :2 * F], AF.Copy)
    RTRIG = T("RTRIG", 2, F32)
    V.reciprocal_approx_fast(out=RTRIG[:], in_=TRIGF[:])
    # RT4 = [Rcr | Rsr | -Rsr | Rcr] fp16, clamped to +-RCL
    RT4 = T("RT4", 4)
    ts(out=_ap(RT4, 0, [[3 * F, 2], [1, F]]),
       in0=_ap(RTRIG, 0, [[0, 2], [1, F]]), scalar1=RCL, scalar2=-RCL,
       op0=OP.min, op1=OP.max)
    ts(out=RT4[:, F:2 * F], in0=RTRIG[:, F:], scalar1=RCL, scalar2=-RCL,
       op0=OP.min, op1=OP.max)
    ts(out=RT4[:, 2 * F:3 * F], in0=RT4[:, F:2 * F], scalar1=-1.0,
       scalar2=None, op0=OP.mult)
    # RWH16B = [2/w1 | 2/h1 | 2/w2 | 2/h2] (recips of half-extents)
    RWHF = T("RWHF", 4, F32)
    V.reciprocal_approx_fast(out=RWHF[:, :2 * F], in_=TX1[:, 2 * F:])
    V.reciprocal_approx_fast(out=RWHF[:, 2 * F:], in_=TX2[:, 2 * F:])
    RWH16 = T("RWH16", 4)
    ts(out=RWH16[:], in0=RWHF[:], scalar1=2.0, scalar2=None, op0=OP.mult)
    # WH16B = [w1/2 | h1/2 | w2/2 | h2/2] fp16
    WH16 = T("WH16", 4)
    ts(out=WH16[:, :2 * F], in0=TX1[:, 2 * F:], scalar1=0.5, scalar2=None,
       op0=OP.mult)
    ts(out=WH16[:, 2 * F:], in0=TX2[:, 2 * F:], scalar1=0.5, scalar2=None,
       op0=OP.mult)

    # ---------------- transforms (fp16) ----------------
    DXY = T("DXY", 2)
    tt(DXY[:], TX1[:, :2 * F], TX2[:, :2 * F], OP.subtract)  # [dx | dy]
    dxy_sw = _ap(DXY, F, [[-F, 2], [1, F]])                  # [dy | dx]
    # P12 = [c2 dx | s2 dy | c2 dy | -s2 dx]
    P12 = T("P12", 4)
    tt(P12[:, :2 * F], SC3[:, :2 * F], DXY[:], OP.mult)
    tt(q(P12[:, 2 * F:]), _ap(SC3, 0, [[2 * F, 2], [1, F]]), dxy_sw, OP.mult)
    # TC16 = [tx | cx | ty | cy]
    TC16 = T("TC16", 4)
    txty = _ap(TC16, 0, [[2 * F, 2], [1, F]])    # [tx | ty] dest/src view
    tt(txty, _ap(P12, 0, [[2 * F, 2], [1, F]]),
       _ap(P12, F, [[2 * F, 2], [1, F]]), OP.add)
    # P34 = [cr tx | sr ty | cr ty | -sr tx]
    tytx = _ap(TC16, 2 * F, [[-2 * F, 2], [1, F]])  # [ty | tx]
    P34 = T("P34", 4, tag="P12")
    tt(q(P34[:, :2 * F]), q(TRIG4[:, :2 * F]), txty, OP.mult)
    tt(q(P34[:, 2 * F:]), _ap(TRIG4, 0, [[2 * F, 2], [1, F]]), tytx, OP.mult)
    QQ = T("QQ", 2)
    tt(q(QQ[:]), _ap(P34, 0, [[2 * F, 2], [1, F]]),
       _ap(P34, F, [[2 * F, 2], [1, F]]), OP.add)  # [-cx | -cy]
    cxcy = _ap(TC16, F, [[2 * F, 2], [1, F]])    # [cx | cy] dest view
    ts(out=cxcy, in0=q(QQ[:]), scalar1=-1.0, scalar2=None, op0=OP.mult)

    # ---------------- per-pass pairs (merged w=4 ops) ----------------
    # RP = [p1: (rpx pair | rpy pair) | p2: (...)]; rp*-pair = signed 2/D'
    # pass1: rpx = (-2Rh1 Rsr, 2Rw1 Rcr), rpy = (2Rh1 Rcr, 2Rw1 Rsr)
    # pass2: rpx = (2Rh2 Rsr, 2Rw2 Rcr),  rpy = (2Rh2 Rcr, -2Rw2 Rsr)
    RP = T("RP", 8)
    rw1rep = _ap(RWH16, F, [[0, 2], [-F, 2], [1, F]])    # [2/h1 2/w1] x2
    rw2rep = _ap(RWH16, 3 * F, [[0, 2], [-F, 2], [1, F]])
    rt_p1 = _ap(RT4, 2 * F, [[-2 * F, 2], [F, 2], [1, F]])  # [-Rsr Rcr | Rcr Rsr]
    rt_p2 = _ap(RT4, F, [[2 * F, 2], [-F, 2], [1, F]])      # [Rsr Rcr | Rcr -Rsr]
    tt(RP[:, :4 * F].rearrange("p (i j f) -> p i j f", i=2, j=2),
       rw1rep, rt_p1, OP.mult)
    tt(RP[:, 4 * F:].rearrange("p (i j f) -> p i j f", i=2, j=2),
       rw2rep, rt_p2, OP.mult)
    # OFF = [p1: (offx pair | offy pair) | p2: (...)]; half-dir midpoint offsets
    # pass1: offx = (w1/2 cr, -h1/2 sr), offy = (w1/2 sr, h1/2 cr)
    # pass2: offx = (w2/2 cr, h2/2 sr),  offy = (-w2/2 sr, h2/2 cr)
    OFF = T("OFF", 8)
    wh1rep = _ap(WH16, 0, [[0, 2], [F, 2], [1, F]])      # [w1/2 h1/2] x2
    wh2rep = _ap(WH16, 2 * F, [[0, 2], [F, 2], [1, F]])
    tg_p1 = _ap(TRIG4, 0, [[F, 2], [2 * F, 2], [1, F]])  # [cr -sr | sr cr]
    tg_p2 = _ap(TRIG4, 0, [[2 * F, 2], [F, 2], [1, F]])  # [cr sr | -sr cr]
    tt(OFF[:, :4 * F].rearrange("p (i j f) -> p i j f", i=2, j=2),
       wh1rep, tg_p1, OP.mult)
    tt(OFF[:, 4 * F:].rearrange("p (i j f) -> p i j f", i=2, j=2),
       wh2rep, tg_p2, OP.mult)

    # slot layout in [P, 8F] tiles: [p1+(2F) | p2+(2F) | p1-(2F) | p2-(2F)]
    tcx = _ap(TC16, 0, [[F, 2], [0, 2], [1, F]])     # [tx tx cx cx]
    tcy = _ap(TC16, 2 * F, [[F, 2], [0, 2], [1, F]])  # [ty ty cy cy]
    offx4 = _ap(OFF, 0, [[4 * F, 2], [F, 2], [1, F]])      # [offx1 | offx2]
    offy4 = _ap(OFF, 2 * F, [[4 * F, 2], [F, 2], [1, F]])  # [offy1 | offy2]
    UX = T("UX", 8)
    tt(UX[:, :4 * F].rearrange("p (i j f) -> p i j f", i=2, j=2),
       tcx, offx4, OP.add)
    tt(UX[:, 4 * F:].rearrange("p (i j f) -> p i j f", i=2, j=2),
       tcx, offx4, OP.subtract)
    UY = T("UY", 8)
    tt(UY[:, :4 * F].rearrange("p (i j f) -> p i j f", i=2, j=2),
       tcy, offy4, OP.add)
    tt(UY[:, 4 * F:].rearrange("p (i j f) -> p i j f", i=2, j=2),
       tcy, offy4, OP.subtract)
    # mneg = u * r  (r repeats over the +/- halves)
    rpx_rep = _ap(RP, 0, [[0, 2], [4 * F, 2], [1, 2 * F]])
    rpy_rep = _ap(RP, 2 * F, [[0, 2], [4 * F, 2], [1, 2 * F]])
    MX = T("MX", 8)
    tt(MX[:].rearrange("p (r s f) -> p r s f", r=2, s=2),
       UX[:].rearrange("p (r s f) -> p r s f", r=2, s=2), rpx_rep, OP.mult)
    MY = T("MY", 8)
    tt(MY[:].rearrange("p (r s f) -> p r s f", r=2, s=2),
       UY[:].rearrange("p (r s f) -> p r s f", r=2, s=2), rpy_rep, OP.mult)
    # slab half-widths: sigma = W''_other * |r|
    w2rep = _ap(WH16, 2 * F, [[-2 * F, 2], [0, 2], [1, F]])  # [w2/2 w2/2 w1/2 w1/2]
    h2rep = _ap(WH16, 3 * F, [[-2 * F, 2], [0, 2], [1, F]])  # [h2/2 h2/2 h1/2 h1/2]
    rpx_pt = _ap(RP, 0, [[4 * F, 2], [F, 2], [1, F]])
    rpy_pt = _ap(RP, 2 * F, [[4 * F, 2], [F, 2], [1, F]])
    PTX = T("PTX", 4)
    tt(PTX[:].rearrange("p (i j f) -> p i j f", i=2, j=2), w2rep,
       rpx_pt, OP.mult)
    PTY = T("PTY", 4)
    tt(PTY[:].rearrange("p (i j f) -> p i j f", i=2, j=2), h2rep,
       rpy_pt, OP.mult)
    SQX = T("SQX", 4, tag="OFF")
    ts(out=SQX[:].bitcast(U16), in0=PTX[:].bitcast(U16), scalar1=0x7FFF,
       scalar2=None, op0=OP.bitwise_and)
    SQY = T("SQY", 4, tag="RP")
    ts(out=SQY[:].bitcast(U16), in0=PTY[:].bitcast(U16), scalar1=0x7FFF,
       scalar2=None, op0=OP.bitwise_and)
    # combos: ax = sigma - mneg, bx = sigma + mneg
    sqx_rep = _ap(SQX, 0, [[0, 2], [1, 4 * F]])
    sqy_rep = _ap(SQY, 0, [[0, 2], [1, 4 * F]])
    AX = T("AX", 8, tag="UX")
    tt(q(AX[:]), sqx_rep, q(MX[:]), OP.subtract)
    BX = T("BX", 8)
    tt(q(BX[:]), sqx_rep, q(MX[:]), OP.add)
    AY = T("AY", 8, tag="UY")
    tt(q(AY[:]), sqy_rep, q(MY[:]), OP.subtract)
    BY = T("BY", 8, tag="MX")
    tt(q(BY[:]), sqy_rep, q(MY[:]), OP.add)
    NL = T("NL", 8, tag="MY")
    tt(NL[:], AX[:], AY[:], OP.min)
    HI = T("HI", 8, tag="UX")
    tt(HI[:], BX[:], BY[:], OP.min)
    NLC = T("NLC", 8, tag="BX")
    ts(out=NLC[:], in0=NL[:], scalar1=1.0, scalar2=None, op0=OP.min)
    HIC = T("HIC", 8, tag="UY")
    ts(out=HIC[:], in0=HI[:], scalar1=1.0, scalar2=None, op0=OP.min)
    DS = T("DS", 8, tag="MY")
    tt(DS[:], NLC[:], HIC[:], OP.add)
    RDT = T("RDT", 8, tag="UX")
    ts(out=RDT[:], in0=DS[:], scalar1=0.0, scalar2=None, op0=OP.max)

    # ---------------- combine ----------------
    # RDT slots: [w+1 h+1 (2F) | w+2 h+2 | w-1 h-1 | w-2 h-2]
    # DS12 = [d_w | -d_h | S1 | S2];  d_w = w+1 - w-1, -d_h = h+1 - h-1
    DS12 = T("DS12", 4)
    tt(DS12[:, :2 * F], RDT[:, :2 * F], RDT[:, 4 * F:6 * F], OP.subtract)
    SSQ = T("SSQ", 4, tag="BX")
    G.tensor_tensor(q(SSQ[:], 2).rearrange("p r (j f) -> p r j f", j=2),
       q(RDT[:, :4 * F], 2).rearrange("p r (j f) -> p r j f", j=2),
       q(RDT[:, 4 * F:], 2).rearrange("p r (j f) -> p r j f", j=2), OP.add)
    ssA = _ap(SSQ, 0, [[2 * F, 2], [1, F]])
    ssB = _ap(SSQ, F, [[2 * F, 2], [1, F]])
    G.tensor_tensor(q(DS12[:, 2 * F:]), ssA, ssB, OP.add)     # [S1 | S2]
    # CW = [crh/2 | -crw/2 | wh1/4 | wh2/4]
    wh_sw1 = _ap(WH16, F, [[-F, 2], [1, F]])     # [h1/2 | w1/2]
    CW = T("CW", 4, tag="P12")
    tt(q(CW[:, :2 * F]), wh_sw1, q(QQ[:]), OP.mult)
    whA = _ap(WH16, 0, [[2 * F, 2], [1, F]])
    whB = _ap(WH16, F, [[2 * F, 2], [1, F]])
    G.tensor_tensor(q(CW[:, 2 * F:]), whA, whB, OP.mult)
    # XX4 = [crh/2 d_w | crw/2 d_h | wh1 S1/4 | wh2 S2/4]
    XX4 = T("XX4", 4)
    tt(XX4[:], CW[:], DS12[:], OP.mult)
    xdA = _ap(XX4, 0, [[2 * F, 2], [1, F]])
    xdB = _ap(XX4, F, [[2 * F, 2], [1, F]])
    XDH = T("XDH", 2, tag="QQ")
    tt(q(XDH[:]), xdA, xdB, OP.add)              # [G1/2 | HS/4]
    GH = T("GH", 1)
    tt(GH[:], XDH[:, :F], XDH[:, F:], OP.add)    # G/2
    ABF = T("ABF", 1)
    ts(out=ABF[:].bitcast(U16), in0=GH[:].bitcast(U16), scalar1=0x7FFF,
       scalar2=None, op0=OP.bitwise_and)         # |G|/2 = 2*area, fp16
    SW = T("SW", 1)
    tt(SW[:], CW[:, 2 * F:3 * F], CW[:, 3 * F:], OP.add)  # (wh1+wh2)/4
    SW8 = T("SW8", 1, F32)
    A.activation(SW8[:], SW[:], AF.Copy, scale=8.0)
    U4 = T("U4", 1, F32)
    tt(U4[:], SW8[:], ABF[:], OP.subtract)       # 2(a1+a2) - 2*area
    RC = T("RC", 1, F32)
    V.reciprocal_approx_fast(out=RC[:], in_=U4[:])
    IOU = T("IOU", 1, F32)
    tt(IOU[:], ABF[:], RC[:], OP.mult)
    ioudst = bass.AP(iou.ap().tensor, iou.ap().offset, [[F, P], [1, F]])
    nc.sync.dma_start(ioudst, IOU[:])


def _get_program():
    key = ("prog", os.environ.get("KREPEAT", "1"))
    if key not in _CACHE:
        _CACHE[key] = _build_program()
    return _CACHE[key]


def kernel(box1, box2, trace=False):
    global LAST_RESULTS
    b1 = np.asarray(box1, dtype=np.float32)
    b2 = np.asarray(box2, dtype=np.float32)
    B, N, C = b1.shape
    Tn = B * N
    assert Tn == NCORES * S and C == 5, (b1.shape,)
    b1f = np.ascontiguousarray(
        b1.reshape(NCORES, P, F, 5).transpose(0, 1, 3, 2)).reshape(NCORES, P, 5 * F)
    b2f = np.ascontiguousarray(
        b2.reshape(NCORES, P, F, 5).transpose(0, 1, 3, 2)).reshape(NCORES, P, 5 * F)
    in_maps = [{"b1": b1f[i], "b2": b2f[i]} for i in range(NCORES)]
    nc = _get_program()
    res = run_bass_kernel_spmd(nc, in_maps, list(range(NCORES)), trace=trace)
    LAST_RESULTS = res
    out = np.concatenate([res.results[i]["iou"] for i in range(NCORES)])
    return out.reshape(B, N)


if __name__ == "__main__":
    from concourse.bass_interp import CoreSim

    nc = _get_program()
    print("program built; instructions:",
          sum(len(bb.instructions) for bb in nc.main_func.blocks))
    d = np.load(os.path.join(os.path.dirname(os.path.abspath(__file__)),
                             "ref_cache.npz"))
    b1 = d["box1"].reshape(-1, 5)[:S]
    b2 = d["box2"].reshape(-1, 5)[:S]
    exp = d["expected"].reshape(-1)[:S]
    sim = CoreSim(nc)
    sim.tensor("b1")[:] = np.ascontiguousarray(
        b1.reshape(P, F, 5).transpose(0, 2, 1)).reshape(P, 5 * F)
    sim.tensor("b2")[:] = np.ascontiguousarray(
        b2.reshape(P, F, 5).transpose(0, 2, 1)).reshape(P, 5 * F)
    sim.simulate()
    got = np.array(sim.tensor("iou"))
    err = got - exp
    l2 = np.linalg.norm(err) / np.linalg.norm(exp)
    print(f"CoreSim vs reference: L2rel {l2:.3e} maxabs {np.abs(err).max():.3e} "
          f"nan {np.isnan(got).sum()}")


# revision 7
# speedup vs baseline: 1.1048x; 1.0156x over previous
"""Trainium2 Bass kernel for differentiable rotated-box IoU (DiffIouRotated).

Full inputs: box1, box2 [4, 131072, 5] f32 (x, y, w, h, alpha).
Output: IoU [4, 131072] f32.

Green's-theorem formulation with edge-midpoint parametrization (see
proto_new.py): each of the 8 box edges contributes
(1/4)cross(mid,D)*Δs⁺, where Δs⁺ is the Liang-Barsky-clipped length in
s ∈ [-1,1] against the other box's slab.  Per-edge cross terms reduce to
two cross products plus Δs sums/differences.

Numerics: fp16 everywhere past the trig (DVE 2-byte tensor_tensor runs
the 2x perf mode, tensor_scalar the 4x mode); f32 only for sin/cos and
the base reciprocals 1/cr, 1/sr, 1/w, 1/h (clamped to ±4096 before the
fp16 convert so no inf-inf can arise).  Measured L2 rel err 1.4e-3 on
the reference data (gate 2e-2).

The two clip passes are processed as merged [128 x 8*512] tiles so most
interval-stage instructions cover both passes at once (w=8 ops).

Sharding: data-parallel, 65536 pairs/core as [128 part x 512 free],
field-major SBUF layout.
"""

import os
import sys

import numpy as np

if "/opt/trn_rl_repo" not in sys.path:
    sys.path.insert(0, "/opt/trn_rl_repo")

import concourse.bass as bass
import concourse.bacc as bacc
import concourse.mybir as mybir
from concourse.bass_utils import run_bass_kernel_spmd
from concourse.tile import TileContext

F32 = mybir.dt.float32
F16 = mybir.dt.float16
U16 = mybir.dt.uint16
U32 = mybir.dt.uint32
OP = mybir.AluOpType
AF = mybir.ActivationFunctionType

NCORES = 8
P = 128
S = 65536
F = S // P           # 512
PI = float(np.pi)
RCL = 4096.0         # clamp for 1/sin, 1/cos before fp16 convert

_CACHE = {}
LAST_RESULTS = None


def _build_program():
    nc = bacc.Bacc("TRN2", target_bir_lowering=False, debug=False,
                   num_devices=NCORES)
    _ct = nc.alloc_sbuf_tensor("const-f32-halfpi", [P, 1], F32)
    nc.gpsimd.memset(_ct.ap(), PI / 2)
    nc.const_aps.aps[(F32, PI / 2)] = _ct.ap()
    nc.all_engine_barrier()

    b1 = nc.dram_tensor("b1", [P, 5 * F], F32, kind="ExternalInput")
    b2 = nc.dram_tensor("b2", [P, 5 * F], F32, kind="ExternalInput")
    iou = nc.dram_tensor("iou", [S], F32, kind="ExternalOutput")

    repeat = int(os.environ.get("KREPEAT", "1"))
    with TileContext(nc) as tc:
        with tc.tile_pool(name="pool", bufs=1) as pool:
            if repeat > 1:
                with tc.For_i(0, repeat, 1):
                    _emit(nc, pool, b1, b2, iou)
            else:
                _emit(nc, pool, b1, b2, iou)
    nc.compile()
    return nc


def _ap(t, off, dims):
    return bass.AP(t.tensor, t.offset + off, [t.ap[0]] + dims)


def _emit(nc, pool, b1, b2, iou):
    V, G, A = nc.vector, nc.gpsimd, nc.scalar
    tt = V.tensor_tensor
    ts = V.tensor_scalar

    def T(name, w=1, dt=F16, tag=None):
        return pool.tile([P, w * F], dt, name=name, tag=(tag or name))

    def q(t, r=2):
        return t.rearrange("p (r f) -> p r f", r=r)

    # ---------------- input DMAs ----------------
    TA1, TA2 = T("TA1", 1, F32), T("TA2", 1, F32)
    TX1, TX2 = T("TX1", 4, F32), T("TX2", 4, F32)

    b1v = b1.ap().flatten().rearrange("(p q) -> p q", p=P)
    b2v = b2.ap().flatten().rearrange("(p q) -> p q", p=P)
    nc.sync.dma_start(TA1[:], b1v[:, 4 * F:])
    nc.sync.dma_start(TA2[:], b2v[:, 4 * F:])
    nc.sync.dma_start(TX1[:], b1v[:, :4 * F])
    nc.sync.dma_start(TX2[:], b2v[:, :4 * F])

    # ---------------- trig (Act) ----------------
    DA = T("DA", 1, F32)
    tt(DA[:], TA1[:], TA2[:], OP.subtract)
    ADA = T("ADA", 1, F32)
    A.activation(ADA[:], DA[:], AF.Abs)
    # TRIG4 = [cr | sr | -sr | cr] fp16 ; SC3 = [c2 | s2 | -s2] fp16
    TRIG4 = T("TRIG4", 4)
    cr_both = _ap(TRIG4, 0, [[3 * F, 2], [1, F]])
    A.activation(cr_both, _ap(ADA, 0, [[0, 2], [1, F]]), AF.Sin,
                 bias=PI / 2, scale=-1.0)
    A.activation(TRIG4[:, F:2 * F], DA[:], AF.Sin)
    A.activation(TRIG4[:, 2 * F:3 * F], DA[:], AF.Sin, scale=-1.0)
    SC3 = T("SC3", 3)
    A.activation(SC3[:, :F], TA2[:], AF.Sin, bias=PI / 2, scale=-1.0)
    A.activation(SC3[:, F:2 * F], TA2[:], AF.Sin)
    A.activation(SC3[:, 2 * F:], TA2[:], AF.Sin, scale=-1.0)

    # trig-independent DVE work, emitted here to overlap the Act trig chain
    # RWH16B = [2/w1 | 2/h1 | 2/w2 | 2/h2] (recips of half-extents)
    RWHF = T("RWHF", 4, F32)
    V.reciprocal_approx_fast(out=RWHF[:, :2 * F], in_=TX1[:, 2 * F:])
    V.reciprocal_approx_fast(out=RWHF[:, 2 * F:], in_=TX2[:, 2 * F:])
    RWH16 = T("RWH16", 4)
    ts(out=RWH16[:], in0=RWHF[:], scalar1=2.0, scalar2=None, op0=OP.mult)
    # WH16B = [w1/2 | h1/2 | w2/2 | h2/2] fp16
    WH16 = T("WH16", 4)
    ts(out=WH16[:, :2 * F], in0=TX1[:, 2 * F:], scalar1=0.5, scalar2=None,
       op0=OP.mult)
    ts(out=WH16[:, 2 * F:], in0=TX2[:, 2 * F:], scalar1=0.5, scalar2=None,
       op0=OP.mult)
    DXY = T("DXY", 2)
    tt(DXY[:], TX1[:, :2 * F], TX2[:, :2 * F], OP.subtract)  # [dx | dy]

    # ---------------- reciprocals (f32 core) ----------------
    TRIGF = T("TRIGF", 2, F32)
    A.activation(TRIGF[:], TRIG4[:, :2 * F], AF.Copy)
    RTRIG = T("RTRIG", 2, F32)
    V.reciprocal_approx_fast(out=RTRIG[:], in_=TRIGF[:])
    # RT4 = [Rcr | Rsr | -Rsr | Rcr] fp16, clamped to +-RCL
    RT4 = T("RT4", 4)
    ts(out=_ap(RT4, 0, [[3 * F, 2], [1, F]]),
       in0=_ap(RTRIG, 0, [[0, 2], [1, F]]), scalar1=RCL, scalar2=-RCL,
       op0=OP.min, op1=OP.max)
    ts(out=RT4[:, F:2 * F], in0=RTRIG[:, F:], scalar1=RCL, scalar2=-RCL,
       op0=OP.min, op1=OP.max)
    ts(out=RT4[:, 2 * F:3 * F], in0=RT4[:, F:2 * F], scalar1=-1.0,
       scalar2=None, op0=OP.mult)

    # ---------------- transforms (fp16) ----------------
    dxy_sw = _ap(DXY, F, [[-F, 2], [1, F]])                  # [dy | dx]
    # P12 = [c2 dx | s2 dy | c2 dy | -s2 dx]
    P12 = T("P12", 4)
    tt(P12[:, :2 * F], SC3[:, :2 * F], DXY[:], OP.mult)
    tt(q(P12[:, 2 * F:]), _ap(SC3, 0, [[2 * F, 2], [1, F]]), dxy_sw, OP.mult)
    # TC16 = [tx | cx | ty | cy]
    TC16 = T("TC16", 4)
    txty = _ap(TC16, 0, [[2 * F, 2], [1, F]])    # [tx | ty] dest/src view
    tt(txty, _ap(P12, 0, [[2 * F, 2], [1, F]]),
       _ap(P12, F, [[2 * F, 2], [1, F]]), OP.add)
    # P34 = [cr tx | sr ty | cr ty | -sr tx]
    tytx = _ap(TC16, 2 * F, [[-2 * F, 2], [1, F]])  # [ty | tx]
    P34 = T("P34", 4, tag="P12")
    tt(q(P34[:, :2 * F]), q(TRIG4[:, :2 * F]), txty, OP.mult)
    tt(q(P34[:, 2 * F:]), _ap(TRIG4, 0, [[2 * F, 2], [1, F]]), tytx, OP.mult)
    QQ = T("QQ", 2)
    tt(q(QQ[:]), _ap(P34, 0, [[2 * F, 2], [1, F]]),
       _ap(P34, F, [[2 * F, 2], [1, F]]), OP.add)  # [-cx | -cy]
    cxcy = _ap(TC16, F, [[2 * F, 2], [1, F]])    # [cx | cy] dest view
    ts(out=cxcy, in0=q(QQ[:]), scalar1=-1.0, scalar2=None, op0=OP.mult)

    # ---------------- per-pass pairs (merged w=4 ops) ----------------
    # RP = [p1: (rpx pair | rpy pair) | p2: (...)]; rp*-pair = signed 2/D'
    # pass1: rpx = (-2Rh1 Rsr, 2Rw1 Rcr), rpy = (2Rh1 Rcr, 2Rw1 Rsr)
    # pass2: rpx = (2Rh2 Rsr, 2Rw2 Rcr),  rpy = (2Rh2 Rcr, -2Rw2 Rsr)
    RP = T("RP", 8)
    rw1rep = _ap(RWH16, F, [[0, 2], [-F, 2], [1, F]])    # [2/h1 2/w1] x2
    rw2rep = _ap(RWH16, 3 * F, [[0, 2], [-F, 2], [1, F]])
    rt_p1 = _ap(RT4, 2 * F, [[-2 * F, 2], [F, 2], [1, F]])  # [-Rsr Rcr | Rcr Rsr]
    rt_p2 = _ap(RT4, F, [[2 * F, 2], [-F, 2], [1, F]])      # [Rsr Rcr | Rcr -Rsr]
    tt(RP[:, :4 * F].rearrange("p (i j f) -> p i j f", i=2, j=2),
       rw1rep, rt_p1, OP.mult)
    tt(RP[:, 4 * F:].rearrange("p (i j f) -> p i j f", i=2, j=2),
       rw2rep, rt_p2, OP.mult)
    # OFF = [p1: (offx pair | offy pair) | p2: (...)]; half-dir midpoint offsets
    # pass1: offx = (w1/2 cr, -h1/2 sr), offy = (w1/2 sr, h1/2 cr)
    # pass2: offx = (w2/2 cr, h2/2 sr),  offy = (-w2/2 sr, h2/2 cr)
    OFF = T("OFF", 8)
    wh1rep = _ap(WH16, 0, [[0, 2], [F, 2], [1, F]])      # [w1/2 h1/2] x2
    wh2rep = _ap(WH16, 2 * F, [[0, 2], [F, 2], [1, F]])
    tg_p1 = _ap(TRIG4, 0, [[F, 2], [2 * F, 2], [1, F]])  # [cr -sr | sr cr]
    tg_p2 = _ap(TRIG4, 0, [[2 * F, 2], [F, 2], [1, F]])  # [cr sr | -sr cr]
    tt(OFF[:, :4 * F].rearrange("p (i j f) -> p i j f", i=2, j=2),
       wh1rep, tg_p1, OP.mult)
    tt(OFF[:, 4 * F:].rearrange("p (i j f) -> p i j f", i=2, j=2),
       wh2rep, tg_p2, OP.mult)

    # slot layout in [P, 8F] tiles: [p1+(2F) | p2+(2F) | p1-(2F) | p2-(2F)]
    tcx = _ap(TC16, 0, [[F, 2], [0, 2], [1, F]])     # [tx tx cx cx]
    tcy = _ap(TC16, 2 * F, [[F, 2], [0, 2], [1, F]])  # [ty ty cy cy]
    offx4 = _ap(OFF, 0, [[4 * F, 2], [F, 2], [1, F]])      # [offx1 | offx2]
    offy4 = _ap(OFF, 2 * F, [[4 * F, 2], [F, 2], [1, F]])  # [offy1 | offy2]
    UX = T("UX", 8)
    tt(UX[:, :4 * F].rearrange("p (i j f) -> p i j f", i=2, j=2),
       tcx, offx4, OP.add)
    tt(UX[:, 4 * F:].rearrange("p (i j f) -> p i j f", i=2, j=2),
       tcx, offx4, OP.subtract)
    UY = T("UY", 8)
    tt(UY[:, :4 * F].rearrange("p (i j f) -> p i j f", i=2, j=2),
       tcy, offy4, OP.add)
    tt(UY[:, 4 * F:].rearrange("p (i j f) -> p i j f", i=2, j=2),
       tcy, offy4, OP.subtract)
    # mneg = u * r  (r repeats over the +/- halves)
    rpx_rep = _ap(RP, 0, [[0, 2], [4 * F, 2], [1, 2 * F]])
    rpy_rep = _ap(RP, 2 * F, [[0, 2], [4 * F, 2], [1, 2 * F]])
    MX = T("MX", 8)
    tt(MX[:].rearrange("p (r s f) -> p r s f", r=2, s=2),
       UX[:].rearrange("p (r s f) -> p r s f", r=2, s=2), rpx_rep, OP.mult)
    MY = T("MY", 8)
    tt(MY[:].rearrange("p (r s f) -> p r s f", r=2, s=2),
       UY[:].rearrange("p (r s f) -> p r s f", r=2, s=2), rpy_rep, OP.mult)
    # slab half-widths: sigma = W''_other * |r|
    w2rep = _ap(WH16, 2 * F, [[-2 * F, 2], [0, 2], [1, F]])  # [w2/2 w2/2 w1/2 w1/2]
    h2rep = _ap(WH16, 3 * F, [[-2 * F, 2], [0, 2], [1, F]])  # [h2/2 h2/2 h1/2 h1/2]
    rpx_pt = _ap(RP, 0, [[4 * F, 2], [F, 2], [1, F]])
    rpy_pt = _ap(RP, 2 * F, [[4 * F, 2], [F, 2], [1, F]])
    PTX = T("PTX", 4)
    tt(PTX[:].rearrange("p (i j f) -> p i j f", i=2, j=2), w2rep,
       rpx_pt, OP.mult)
    PTY = T("PTY", 4)
    tt(PTY[:].rearrange("p (i j f) -> p i j f", i=2, j=2), h2rep,
       rpy_pt, OP.mult)
    SQX = T("SQX", 4, tag="OFF")
    ts(out=SQX[:].bitcast(U16), in0=PTX[:].bitcast(U16), scalar1=0x7FFF,
       scalar2=None, op0=OP.bitwise_and)
    SQY = T("SQY", 4, tag="RP")
    ts(out=SQY[:].bitcast(U16), in0=PTY[:].bitcast(U16), scalar1=0x7FFF,
       scalar2=None, op0=OP.bitwise_and)
    # combos: ax = sigma - mneg, bx = sigma + mneg
    sqx_rep = _ap(SQX, 0, [[0, 2], [1, 4 * F]])
    sqy_rep = _ap(SQY, 0, [[0, 2], [1, 4 * F]])
    AX = T("AX", 8, tag="UX")
    tt(q(AX[:]), sqx_rep, q(MX[:]), OP.subtract)
    BX = T("BX", 8)
    tt(q(BX[:]), sqx_rep, q(MX[:]), OP.add)
    AY = T("AY", 8, tag="UY")
    tt(q(AY[:]), sqy_rep, q(MY[:]), OP.subtract)
    BY = T("BY", 8, tag="MX")
    tt(q(BY[:]), sqy_rep, q(MY[:]), OP.add)
    NL = T("NL", 8, tag="MY")
    tt(NL[:], AX[:], AY[:], OP.min)
    HI = T("HI", 8, tag="UX")
    tt(HI[:], BX[:], BY[:], OP.min)
    NLC = T("NLC", 8, tag="BX")
    ts(out=NLC[:], in0=NL[:], scalar1=1.0, scalar2=None, op0=OP.min)
    HIC = T("HIC", 8, tag="UY")
    ts(out=HIC[:], in0=HI[:], scalar1=1.0, scalar2=None, op0=OP.min)
    DS = T("DS", 8, tag="MY")
    tt(DS[:], NLC[:], HIC[:], OP.add)
    RDT = T("RDT", 8, tag="UX")
    ts(out=RDT[:], in0=DS[:], scalar1=0.0, scalar2=None, op0=OP.max)

    # ---------------- combine ----------------
    # RDT slots: [w+1 h+1 (2F) | w+2 h+2 | w-1 h-1 | w-2 h-2]
    # DS12 = [d_w | -d_h | S1 | S2];  d_w = w+1 - w-1, -d_h = h+1 - h-1
    DS12 = T("DS12", 4)
    tt(DS12[:, :2 * F], RDT[:, :2 * F], RDT[:, 4 * F:6 * F], OP.subtract)
    SSQ = T("SSQ", 4, tag="BX")
    G.tensor_tensor(q(SSQ[:], 2).rearrange("p r (j f) -> p r j f", j=2),
       q(RDT[:, :4 * F], 2).rearrange("p r (j f) -> p r j f", j=2),
       q(RDT[:, 4 * F:], 2).rearrange("p r (j f) -> p r j f", j=2), OP.add)
    ssA = _ap(SSQ, 0, [[2 * F, 2], [1, F]])
    ssB = _ap(SSQ, F, [[2 * F, 2], [1, F]])
    G.tensor_tensor(q(DS12[:, 2 * F:]), ssA, ssB, OP.add)     # [S1 | S2]
    # CW = [crh/2 | -crw/2 | wh1/4 | wh2/4]
    wh_sw1 = _ap(WH16, F, [[-F, 2], [1, F]])     # [h1/2 | w1/2]
    CW = T("CW", 4, tag="P12")
    tt(q(CW[:, :2 * F]), wh_sw1, q(QQ[:]), OP.mult)
    whA = _ap(WH16, 0, [[2 * F, 2], [1, F]])
    whB = _ap(WH16, F, [[2 * F, 2], [1, F]])
    G.tensor_tensor(q(CW[:, 2 * F:]), whA, whB, OP.mult)
    # XX4 = [crh/2 d_w | crw/2 d_h | wh1 S1/4 | wh2 S2/4]
    XX4 = T("XX4", 4)
    tt(XX4[:], CW[:], DS12[:], OP.mult)
    xdA = _ap(XX4, 0, [[2 * F, 2], [1, F]])
    xdB = _ap(XX4, F, [[2 * F, 2], [1, F]])
    XDH = T("XDH", 2, tag="QQ")
    tt(q(XDH[:]), xdA, xdB, OP.add)              # [G1/2 | HS/4]
    GH = T("GH", 1)
    tt(GH[:], XDH[:, :F], XDH[:, F:], OP.add)    # G/2
    ABF = T("ABF", 1)
    ts(out=ABF[:].bitcast(U16), in0=GH[:].bitcast(U16), scalar1=0x7FFF,
       scalar2=None, op0=OP.bitwise_and)         # |G|/2 = 2*area, fp16
    SW = T("SW", 1)
    tt(SW[:], CW[:, 2 * F:3 * F], CW[:, 3 * F:], OP.add)  # (wh1+wh2)/4
    SW8 = T("SW8", 1, F32)
    A.activation(SW8[:], SW[:], AF.Copy, scale=8.0)
    U4 = T("U4", 1, F32)
    tt(U4[:], SW8[:], ABF[:], OP.subtract)       # 2(a1+a2) - 2*area
    RC = T("RC", 1, F32)
    V.reciprocal_approx_fast(out=RC[:], in_=U4[:])
    IOU = T("IOU", 1, F32)
    tt(IOU[:], ABF[:], RC[:], OP.mult)
    ioudst = bass.AP(iou.ap().tensor, iou.ap().offset, [[F, P], [1, F]])
    nc.sync.dma_start(ioudst, IOU[:])


def _get_program():
    key = ("prog", os.environ.get("KREPEAT", "1"))
    if key not in _CACHE:
        _CACHE[key] = _build_program()
    return _CACHE[key]


def kernel(box1, box2, trace=False):
    global LAST_RESULTS
    b1 = np.asarray(box1, dtype=np.float32)
    b2 = np.asarray(box2, dtype=np.float32)
    B, N, C = b1.shape
    Tn = B * N
    assert Tn == NCORES * S and C == 5, (b1.shape,)
    b1f = np.ascontiguousarray(
        b1.reshape(NCORES, P, F, 5).transpose(0, 1, 3, 2)).reshape(NCORES, P, 5 * F)
    b2f = np.ascontiguousarray(
        b2.reshape(NCORES, P, F, 5).transpose(0, 1, 3, 2)).reshape(NCORES, P, 5 * F)
    in_maps = [{"b1": b1f[i], "b2": b2f[i]} for i in range(NCORES)]
    nc = _get_program()
    res = run_bass_kernel_spmd(nc, in_maps, list(range(NCORES)), trace=trace)
    LAST_RESULTS = res
    out = np.concatenate([res.results[i]["iou"] for i in range(NCORES)])
    return out.reshape(B, N)


if __name__ == "__main__":
    from concourse.bass_interp import CoreSim

    nc = _get_program()
    print("program built; instructions:",
          sum(len(bb.instructions) for bb in nc.main_func.blocks))
    d = np.load(os.path.join(os.path.dirname(os.path.abspath(__file__)),
                             "ref_cache.npz"))
    b1 = d["box1"].reshape(-1, 5)[:S]
    b2 = d["box2"].reshape(-1, 5)[:S]
    exp = d["expected"].reshape(-1)[:S]
    sim = CoreSim(nc)
    sim.tensor("b1")[:] = np.ascontiguousarray(
        b1.reshape(P, F, 5).transpose(0, 2, 1)).reshape(P, 5 * F)
    sim.tensor("b2")[:] = np.ascontiguousarray(
        b2.reshape(P, F, 5).transpose(0, 2, 1)).reshape(P, 5 * F)
    sim.simulate()
    got = np.array(sim.tensor("iou"))
    err = got - exp
    l2 = np.linalg.norm(err) / np.linalg.norm(exp)
    print(f"CoreSim vs reference: L2rel {l2:.3e} maxabs {np.abs(err).max():.3e} "
          f"nan {np.isnan(got).sum()}")
